# revision 10
# baseline (speedup 1.0000x reference)
"""Trainium2 Bass kernel for nn_Encoder (GNN message passing encoder).

Computes, for each node b in a batch:
    h[b]  = concat(mean_j feat[neigh[b, j]], feat[nodes[b]]) @ W.T
    out[b] = relu(layernorm(h[b]))          (torch-style unbiased std, eps on std)
returned as [OUT, B] (transposed).

Strategy (8 NeuronCores, data-parallel over the batch):
  - Gathers use the SWDGE extended instruction dma_gather, which fetches
    thousands of table rows per instruction (994ns fixed + ~0.34ns/descriptor)
    instead of indirect_dma_start's one-128-row-per-instruction floor
    (~1.5us each).  The table is stored fp16 (512B rows) to halve HBM bytes.
  - dma_gather indices are signed int16, so the 100k-row table is laid out as
    4 windows of <=32768 rows, each prefixed with a zero row (local idx 0)
    used for padding.  Row r lives in window w = min(r // 32767, 3) at local
    index r - w*32767 + 1.
  - Batch elements are sorted by their per-window neighbor-count vector and
    dealt to (core, tile, partition) so that the 1024 elements sharing a tile
    index have near-identical counts; per tile each window gets
    max-count columns (zero-row padded), plus one self column per window.
    One dma_gather per (tile-group, window) lands all slots at
    [partition p, column c] in a [128, C, 256] fp16 SBUF buffer.
  - The PE sums each element's neighbor columns (identity-matmul accumulate
    into PSUM[:, 0:256]) and self columns (PSUM[:, 256:512]), transposes the
    combined [128, 512] activations, and applies W^T (1/16 mean fold on the
    neighbor half).  LayerNorm+ReLU run on ACT/DVE as before.
  - The program structure (column counts) is input-dependent but identical
    across cores (per-tile max over the 8 cores); only DRAM contents differ.
"""

import os
import sys

sys.path.insert(0, "/opt/trn_rl_repo")
sys.path.insert(0, "/opt/pypackages")

from contextlib import ExitStack

import numpy as np

import concourse.bass as bass
import concourse.tile as tile
from concourse import bacc, mybir
from concourse.bass_utils import run_bass_kernel_spmd
from concourse.masks import make_identity

# Problem constants (hardcoded; kernel.py must be self-contained).
N_NODES, D, OUT, B, K = 100000, 256, 256, 65536, 16
EPS = 1e-6
NCORES = 8
BLOC = B // NCORES  # 8192 nodes per core
P = 128
NT = BLOC // P  # node-tiles per core (64)

WROWS = 32767  # table rows per index window (int16 max)
NW = 4  # windows covering 100k rows
GROUP = int(os.environ.get("ENC_GROUP", "4"))  # tiles per gather group
DMASCRATCH = int(os.environ.get("ENC_DMASCRATCH", "16384"))
NQUEUES = int(os.environ.get("ENC_QUEUES", "4"))  # SWDGE queues for gathers
GATHER_ONLY = os.environ.get("ENC_GATHER_ONLY", "0") == "1"  # diagnostic
SKIP_GATHER = os.environ.get("ENC_SKIP_GATHER", "0") == "1"  # diagnostic


def _win_of(r):
    return np.minimum(r // WROWS, NW - 1)


def pack_table(feat_table, dt=np.float16):
    """[zero, f[0:32767], zero, f[32767:65534], zero, ...] -> [100004, D]."""
    n = feat_table.shape[0]
    parts = []
    for w in range(NW):
        lo, hi = w * WROWS, min((w + 1) * WROWS, n)
        parts.append(np.zeros((1, D), dt))
        parts.append(np.ascontiguousarray(feat_table[lo:hi]).astype(dt))
    return np.ascontiguousarray(np.concatenate(parts, 0))


def _wbase(w):
    """First row of window w inside the packed table."""
    return sum(1 + min((u + 1) * WROWS, N_NODES) - u * WROWS for u in range(w))


def _wrows(w):
    return 1 + min((w + 1) * WROWS, N_NODES) - w * WROWS


def analyze(nodes, neigh_idx, ncores=NCORES, nt=NT, group=GROUP):
    """Sort the batch, compute the shared per-tile column structure, and pack
    per-core int16 gather-index blobs.

    Returns (struct, idx_blobs, order) where struct drives build_program and
    order maps sorted rank -> original batch index.
    """
    nodes = np.asarray(nodes).astype(np.int64).ravel()
    neigh = np.asarray(neigh_idx).astype(np.int64)
    nb = nodes.shape[0]
    assert nb == ncores * nt * P and neigh.shape == (nb, K)

    wn = _win_of(neigh)  # [nb, K]
    counts = np.stack([(wn == w).sum(1) for w in range(NW)], 1)  # [nb, NW]
    order = np.lexsort(tuple(counts[:, w] for w in range(NW - 1, -1, -1)))
    wself = _win_of(nodes)

    # per local tile t: elements are sorted ranks [t*8*128, (t+1)*8*128)
    cnt = np.zeros((nt, NW), np.int32)
    sflag = np.zeros((nt, NW), np.int32)
    for t in range(nt):
        el = order[t * ncores * P : (t + 1) * ncores * P]
        cnt[t] = counts[el].max(0)
        for w in range(NW):
            sflag[t, w] = int((wself[el] == w).any())
    colcnt = cnt + sflag  # columns per (tile, window)

    ngroups = (nt + group - 1) // group
    # per group: window blocks; per (tile,window): sub-offset inside the block
    ginfo = []  # per group: dict(tiles, gc, w: (woff, cols, ioff, nidx))
    ioff = 0  # int16 columns consumed so far in the idx blob
    for g in range(ngroups):
        tiles = list(range(g * group, min((g + 1) * group, nt)))
        woff, wmeta = 0, {}
        for w in range(NW):
            cols = int(colcnt[tiles, w].sum())
            if cols:
                wmeta[w] = dict(woff=woff, cols=cols, ioff=ioff, nidx=cols * P)
                woff += cols
                ioff += cols * P // 16
        ginfo.append(dict(tiles=tiles, gc=woff, wmeta=wmeta))
    gcmax = max(gi["gc"] for gi in ginfo)
    struct = dict(
        nt=nt, ncores=ncores, group=group, ngroups=ngroups,
        cnt=cnt, sflag=sflag, colcnt=colcnt, ginfo=ginfo,
        idxtot=ioff, gcmax=gcmax,
    )

    # ---- pack per-core idx blobs --------------------------------------
    locs_all = (neigh - wn * WROWS + 1).astype(np.int64)  # local idx of each slot
    sloc_all = (nodes - wself * WROWS + 1).astype(np.int64)
    blobs = []
    for c in range(ncores):
        segs = []
        for gi in ginfo:
            for w, m in gi["wmeta"].items():
                rows = []  # each row: one G column = 128 partition values
                for t in gi["tiles"]:
                    el = order[(t * ncores + c) * P : (t * ncores + c + 1) * P]
                    ne_w = wn[el]  # [128, K]
                    mask = ne_w == w
                    key = np.argsort(~mask, axis=1, kind="stable")
                    locs = np.take_along_axis(
                        np.where(mask, locs_all[el], 0), key, 1
                    )[:, : cnt[t, w]]  # [128, cnt]
                    rows.append(locs.T)  # [cnt, 128]
                    if sflag[t, w]:
                        sl = np.where(wself[el] == w, sloc_all[el], 0)
                        rows.append(sl[None, :])  # [1, 128]
                vals = np.concatenate(rows, 0).ravel()  # i = col*128 + p
                assert vals.shape[0] == m["nidx"]
                seg = vals.reshape(-1, 16).T.astype(np.int16)  # [16, nidx/16]
                segs.append(np.tile(seg, (8, 1)))  # [128, nidx/16]
        blob = np.ascontiguousarray(np.concatenate(segs, 1))
        assert blob.shape == (P, ioff)
        blobs.append(blob)
    return struct, blobs, order


def build_program(struct, apply_gamma_beta=False, loop_iters=1):
    """Build the Bass program for one core (SPMD across cores)."""
    f16 = mybir.dt.float16
    f32 = mybir.dt.float32
    i16 = mybir.dt.int16
    nt = struct["nt"]

    nc = bacc.Bacc(
        "TRN2",
        target_bir_lowering=False,
        debug=False,
        dynamic_dma_scratch_size=DMASCRATCH,
        num_swdge_queues=NQUEUES,
    )
    n_packed = _wbase(NW - 1) + _wrows(NW - 1)
    feat = nc.declare_dram_parameter("feat", [n_packed, D], f16, isOutput=False)
    wt = nc.declare_dram_parameter("wt", [2 * D, OUT], f16, isOutput=False)
    idxb = nc.declare_dram_parameter(
        "idxb", [P, struct["idxtot"]], i16, isOutput=False
    )
    if apply_gamma_beta:
        gamma_b = nc.declare_dram_parameter("gamma_b", [P, OUT], f32, isOutput=False)
        beta_b = nc.declare_dram_parameter("beta_b", [P, OUT], f32, isOutput=False)
    out_d = nc.declare_dram_parameter("out", [P * nt, OUT], f32, isOutput=True)

    with tile.TileContext(nc) as tc, ExitStack() as ctx:
        consts = ctx.enter_context(tc.tile_pool(name="consts", bufs=1))
        pool_g = ctx.enter_context(tc.tile_pool(name="gth", bufs=2))
        pool_c = ctx.enter_context(tc.tile_pool(name="comb", bufs=3))
        pool_f = ctx.enter_context(tc.tile_pool(name="f32s", bufs=3))
        pool_sm = ctx.enter_context(tc.tile_pool(name="small", bufs=4))
        psum_r_pool = ctx.enter_context(tc.tile_pool(name="psumR", bufs=2, space="PSUM"))
        psum_t_pool = ctx.enter_context(tc.tile_pool(name="psumT", bufs=2, space="PSUM"))
        psum_h_pool = ctx.enter_context(tc.tile_pool(name="psumH", bufs=2, space="PSUM"))

        # --- constants ---
        ident32 = consts.tile([P, P], f32)
        make_identity(nc, ident32[:])
        ident = consts.tile([P, P], f16)
        nc.vector.tensor_copy(ident[:], ident32[:])

        wt_sb = consts.tile([P, 4 * OUT], f16)
        for c in range(4):
            nc.sync.dma_start(
                out=wt_sb[:, c * OUT : (c + 1) * OUT],
                in_=wt[c * P : (c + 1) * P, :],
            )
        idx_sb = consts.tile([P, struct["idxtot"]], i16)
        nc.sync.dma_start(out=idx_sb[:], in_=idxb[:])
        if apply_gamma_beta:
            gamma_sb = consts.tile([P, OUT], f32)
            nc.sync.dma_start(out=gamma_sb[:], in_=gamma_b[:])
            beta_sb = consts.tile([P, OUT], f32)
            nc.sync.dma_start(out=beta_sb[:], in_=beta_b[:])

        cnt, sflag, colcnt = struct["cnt"], struct["sflag"], struct["colcnt"]

        def tile_compute(t, gt, gi):
            """Sum neighbor/self columns of tile t, project, layernorm."""
            # column ranges of tile t inside the group buffer
            tiles = gi["tiles"]
            ti = tiles.index(t)
            psum_r = psum_r_pool.tile([P, 2 * D], f32, tag="psum_r")
            chains = {0: [], 1: []}  # 0: neigh cols, 1: self cols
            for w, m in gi["wmeta"].items():
                sub = int(colcnt[tiles[:ti], w].sum())
                base = m["woff"] + sub
                for k in range(int(cnt[t, w])):
                    chains[0].append(base + k)
                if sflag[t, w]:
                    chains[1].append(base + int(cnt[t, w]))
            for half, cols in chains.items():
                for i, col in enumerate(cols):
                    nc.tensor.matmul(
                        psum_r[:, half * D : (half + 1) * D],
                        lhsT=ident[:],
                        rhs=gt[:, col, :],
                        start=(i == 0),
                        stop=(i == len(cols) - 1),
                    )

            comb = pool_c.tile([P, 2 * D], f16, tag="comb")
            nc.vector.tensor_copy(comb[:], psum_r[:])
            psum_t = psum_t_pool.tile([P, 2 * D], f16, tag="psum_t")
            for c in range(4):
                nc.tensor.transpose(
                    psum_t[:, c * P : (c + 1) * P], comb[:, c * P : (c + 1) * P],
                    ident[:],
                )
            combT = pool_c.tile([P, 2 * D], f16, tag="combT")
            nc.vector.tensor_copy(combT[:], psum_t[:])

            psum_h = psum_h_pool.tile([P, OUT], f32, tag="psum_h")
            for c in range(4):
                nc.tensor.matmul(
                    psum_h[:],
                    lhsT=combT[:, c * P : (c + 1) * P],
                    rhs=wt_sb[:, c * OUT : (c + 1) * OUT],
                    start=(c == 0),
                    stop=(c == 3),
                )

            # --- LayerNorm (torch unbiased std, eps added to std) + ReLU ---
            negsum = pool_sm.tile([P, 1], f32, tag="negsum")
            nc.vector.tensor_reduce(
                negsum[:], psum_h[:], mybir.AxisListType.X, mybir.AluOpType.add,
                negate=True,
            )
            negmean = pool_sm.tile([P, 1], f32, tag="negmean")
            nc.vector.tensor_scalar_mul(negmean[:], negsum[:], 1.0 / OUT)
            xc = pool_f.tile([P, OUT], f32, tag="xc")
            nc.scalar.activation(
                xc[:], psum_h[:], mybir.ActivationFunctionType.Identity,
                bias=negmean[:, 0:1],
            )
            sq = pool_f.tile([P, OUT], f32, tag="sq")
            ss = pool_sm.tile([P, 1], f32, tag="ss")
            nc.scalar.activation(
                sq[:], xc[:], mybir.ActivationFunctionType.Square,
                accum_out=ss[:, 0:1],
            )
            sstd = pool_sm.tile([P, 1], f32, tag="sstd")
            nc.scalar.activation(
                sstd[:], ss[:], mybir.ActivationFunctionType.Sqrt,
                scale=1.0 / (OUT - 1),
            )
            seps = pool_sm.tile([P, 1], f32, tag="seps")
            nc.vector.tensor_scalar_add(seps[:], sstd[:], EPS)
            rstd = pool_sm.tile([P, 1], f32, tag="rstd")
            nc.vector.reciprocal(rstd[:], seps[:])

            y = pool_f.tile([P, OUT], f32, tag="y")
            if apply_gamma_beta:
                xg = pool_f.tile([P, OUT], f32, tag="xg")
                nc.vector.tensor_tensor(
                    xg[:], xc[:], gamma_sb[:], mybir.AluOpType.mult
                )
                xgs = pool_f.tile([P, OUT], f32, tag="xgs")
                nc.scalar.activation(
                    xgs[:], xg[:], mybir.ActivationFunctionType.Copy,
                    scale=rstd[:, 0:1],
                )
                yb = pool_f.tile([P, OUT], f32, tag="yb")
                nc.vector.tensor_tensor(
                    yb[:], xgs[:], beta_sb[:], mybir.AluOpType.add
                )
                nc.vector.tensor_scalar_max(y[:], yb[:], 0.0)
            else:
                nc.scalar.activation(
                    y[:], xc[:], mybir.ActivationFunctionType.Relu,
                    scale=rstd[:, 0:1],
                )

            nc.sync.dma_start(out=out_d[t * P : (t + 1) * P, :], in_=y[:])

        def body():
            for gi in struct["ginfo"]:
                gt = pool_g.tile([P, struct["gcmax"], D], f16, tag="gth")
                if not SKIP_GATHER:
                    for w, m in gi["wmeta"].items():
                        nc.gpsimd.dma_gather(
                            gt[:, m["woff"] : m["woff"] + m["cols"], :],
                            feat[_wbase(w) : _wbase(w) + _wrows(w), :],
                            idx_sb[:, m["ioff"] : m["ioff"] + m["nidx"] // 16],
                            m["nidx"],
                            m["nidx"],
                            D,
                            single_packet=False,
                            queue_num=w % NQUEUES,
                        )
                if GATHER_ONLY:
                    # keep pool rotation honest with one cheap consumer per group
                    sink = pool_c.tile([P, 1], f16, tag="sink")
                    nc.vector.tensor_copy(sink[:], gt[:, 0, 0:1])
                else:
                    for t in gi["tiles"]:
                        tile_compute(t, gt, gi)

        if GATHER_ONLY:
            zed = consts.tile([P, OUT], f32)
            nc.vector.memset(zed[:], 0.0)
            for t in range(nt):
                nc.sync.dma_start(out=out_d[t * P : (t + 1) * P, :], in_=zed[:])
        if loop_iters > 1:
            with tc.For_i(0, loop_iters, 1):
                body()
        else:
            body()

    nc.finalize()
    return nc


def prepare(feat_table, W, gamma, beta, nodes, neigh_idx):
    """Host-side: analyze indices, build program, pack per-core inputs."""
    feat_table = np.asarray(feat_table, dtype=np.float32)
    W = np.asarray(W, dtype=np.float32)
    gamma = np.asarray(gamma, dtype=np.float32)
    beta = np.asarray(beta, dtype=np.float32)
    nodes = np.asarray(nodes).astype(np.int64)
    neigh_idx = np.asarray(neigh_idx).astype(np.int64)

    struct, blobs, order = analyze(nodes, neigh_idx)

    # combined = [neigh_mean ; self]  ->  W^T rows 0:D get the 1/16 fold.
    wt_host = np.ascontiguousarray(W.T).astype(np.float32)
    wt_host[:D] *= 1.0 / K
    wt_host = wt_host.astype(np.float16)

    trivial_affine = bool(np.all(gamma == 1.0) and np.all(beta == 0.0))
    apply_gb = not trivial_affine

    nc = build_program(struct, apply_gamma_beta=apply_gb)

    feat_dev = pack_table(feat_table)
    in_maps = []
    for c in range(NCORES):
        m = {"feat": feat_dev, "wt": wt_host, "idxb": blobs[c]}
        if apply_gb:
            m["gamma_b"] = np.ascontiguousarray(
                np.broadcast_to(gamma, (P, OUT))
            ).astype(np.float32)
            m["beta_b"] = np.ascontiguousarray(
                np.broadcast_to(beta, (P, OUT))
            ).astype(np.float32)
        in_maps.append(m)
    return nc, in_maps, order


def assemble(results, order):
    out = np.empty((OUT, B), dtype=np.float32)
    for c in range(NCORES):
        ranks = ((np.arange(NT)[:, None] * NCORES + c) * P
                 + np.arange(P)[None, :]).ravel()
        out[:, order[ranks]] = results[c]["out"].T
    return out


def kernel(feat_table, W, gamma, beta, nodes, neigh_idx):
    nc, in_maps, order = prepare(feat_table, W, gamma, beta, nodes, neigh_idx)
    res = run_bass_kernel_spmd(nc, in_maps, list(range(NCORES)))
    return assemble(res.results, order)
